# revision 1
# baseline (speedup 1.0000x reference)
"""BlockSparseMLP (MoE top-2 routing, 8 experts) — Trainium2 Bass kernel.

Strategy (expert-parallel + two-slot load balancing): every core runs
two SPMD-uniform expert blocks of capacity (capA, capB).  An expert
occupies either two A-slots on two cores (AA), two B-slots (BB), or
one A- plus one B-slot on a single core (AB); taking the 2 largest
experts AA, the 2 smallest BB and the middle 4 AB minimizes
capA + capB (518 slots/core for the target routing vs 546 for plain
one-expert-per-core, vs the 512 perfect-balance floor).  Each core
streams two experts' weights — affordable because all weights are
pre-cast to bf16 on the host (half the fp32 HBM bytes; the PE consumes
bf16 either way).

Host-side (numpy): router (x @ gate_tensor, softmax, top-2,
renormalize), token dispatch (gather + transpose + pre-swizzle into the
SBUF block layout), and the final combine (scatter-add of per-expert
partial outputs in fp32).

Device-side (one SPMD Bass/Tile program on 8 cores), per block
b in {A, B} (two expert slots per core):
   gT = Wg_b.T @ xT_b   (bf16 matmuls, fp32 PSUM accumulate)
   uT = Wu_b.T @ xT_b
   aT = silu(gT) * uT   (bf16 in SBUF)
   dT = Wd_b.T @ aT
   outT = dT * w_b      (combine weights folded in on-chip)

The token axis is the matmul moving dimension (single chunk per block,
<= 512 = PSUM bank limit).  Weights are pre-swizzled per-DMA-block
partition-major so every transfer reads large contiguous chunks.  The
per-expert output travels back as bf16 (adds <4e-4 absmax-rel error;
tolerance 2e-2) and is combined in fp32 on the host.
"""

import os

import numpy as np

T, D, F, E, TOPK = 2048, 2048, 5632, 8, 2
P = 128
KD = D // P     # 16 k-subtiles over D
KF = F // P     # 44 k-subtiles over F
FG = 4          # f-tiles per phase-1 weight DMA block (512 F columns)
NFG = KF // FG  # 11 phase-1 blocks
DG = 2          # d-tiles per phase-2 psum group (256 D columns)
NDG = KD // DG  # 8 phase-2 d-groups
KO2 = 11        # f-subtiles per phase-2 weight DMA block
NFB = KF // KO2  # 4 phase-2 blocks per d-group

_COMPILED = {}   # (capA, capB) -> nc
LAST_RESULT = None  # BassKernelResults of the most recent run (for test.py)


def _token_chunks(cap):
    """Split cap into free-dim chunks each <= 512 (PSUM bank limit)."""
    if cap <= 512:
        return [cap]
    n512, rem = divmod(cap, 512)
    if rem == 0:
        return [512] * n512
    if rem >= 256:
        return [512] * n512 + [rem]
    return [512] * (n512 - 1) + [256 + rem // 2, 256 + rem - rem // 2]


def _build(caps):
    """Build + compile the SPMD Tile program for block capacities `caps`."""
    import concourse.bass as bass  # noqa: F401
    import concourse.mybir as mybir
    import concourse.tile as tile
    from concourse import bacc

    f32 = mybir.dt.float32
    bf16 = mybir.dt.bfloat16
    mult = mybir.AluOpType.mult

    capT = sum(caps)
    offs = [0, caps[0]]
    NB = len(caps)

    nc = bacc.Bacc("TRN2", target_bir_lowering=False, debug=False,
                   enable_asserts=False, num_devices=E)

    xt_d = nc.dram_tensor("xt", [P, KD, capT], bf16, kind="ExternalInput").ap()
    wg_d = nc.dram_tensor("wg", [NB, NFG, P, KD, P * FG], bf16,
                          kind="ExternalInput").ap()
    wu_d = nc.dram_tensor("wu", [NB, NFG, P, KD, P * FG], bf16,
                          kind="ExternalInput").ap()
    wd_d = nc.dram_tensor("wd", [NB, NDG, NFB, P, KO2, P * DG], bf16,
                          kind="ExternalInput").ap()
    wr_d = nc.dram_tensor("wrep", [P, capT], f32, kind="ExternalInput").ap()
    out_d = nc.dram_tensor("out_t", [D, capT], bf16,
                           kind="ExternalOutput").ap()
    scr_d = nc.dram_tensor("scr", [P, 8], f32).ap()   # warm-up sink

    with tile.TileContext(nc) as tc:
        with (
            tc.tile_pool(name="resident", bufs=1) as rpool,
            tc.tile_pool(name="w1", bufs=3) as w1pool,
            tc.tile_pool(name="wd2", bufs=6) as wd2pool,
            tc.tile_pool(name="outp", bufs=4) as outpool,
            tc.tile_pool(name="psum", bufs=2, space="PSUM") as ppool,
        ):
            xt = rpool.tile([P, KD, capT], bf16)
            wrep = rpool.tile([P, capT], f32)
            at = rpool.tile([P, KF, capT], bf16)
            kh = KD // 2

            # Ramp: everything streams on the gpsimd/SWDGE ring (the sync
            # sequencer must stay free for cross-engine coordination, and
            # the ACT FIFO would couple DMA issue to compute progress).
            # SWDGE descriptor generation is ~0.7us per op, serialized on
            # the Q7 — keep the critical prefix (first chain's tokens +
            # first weight slice) down to a few ops.
            nc.sync.dma_start(wrep[:], wr_d)
            nc.gpsimd.dma_start(xt[:, 0:4, :], xt_d[:, 0:4, :])
            pre = {}
            wgb = w1pool.tile([P, KD, P * FG], bf16, tag="wgb", name="wgb_0_0")
            wub = w1pool.tile([P, KD, P * FG], bf16, tag="wub", name="wub_0_0")
            sl0 = slice(0, P)
            nc.gpsimd.dma_start(wgb[:, :kh, sl0], wg_d[0, 0][:, :kh, sl0])
            nc.gpsimd.dma_start(wgb[:, kh:, sl0], wg_d[0, 0][:, kh:, sl0])
            for k0 in range(4, KD, 4):
                nc.gpsimd.dma_start(xt[:, k0:k0 + 4, :], xt_d[:, k0:k0 + 4, :])
            nc.gpsimd.dma_start(wub[:, :kh, sl0], wu_d[0, 0][:, :kh, sl0])
            nc.gpsimd.dma_start(wub[:, kh:, sl0], wu_d[0, 0][:, kh:, sl0])
            for s in range(1, FG):
                sl = slice(s * P, (s + 1) * P)
                nc.gpsimd.dma_start(wgb[:, :, sl], wg_d[0, 0][:, :, sl])
                nc.gpsimd.dma_start(wub[:, :, sl], wu_d[0, 0][:, :, sl])
            pre[0] = (wgb, wub)

            # Warm-up: throwaway matmuls bridge the gap between PE sequencer
            # start (~8us) and first-chain data arrival (~13.4us: Q7 boot +
            # SWDGE descriptor-gen + first transfers), and open the PE HAM
            # clock-gate (1.2 -> 2.4 GHz) before real work: 12 cold MMs
            # ~= 5.1us, tuned to end at data arrival.
            warm = rpool.tile([P, 512], bf16)
            nc.vector.memset(warm[:], 0.0)
            wps = ppool.tile([P, 512], f32, tag="pg", name="warm_ps")
            for i in range(12):
                nc.tensor.matmul(wps[:], warm[:, :P], warm[:],
                                 start=(i == 0), stop=(i == 11))
            wout = rpool.tile([P, 8], f32)
            nc.vector.tensor_copy(out=wout[:], in_=wps[:, :8])
            nc.sync.dma_start(scr_d[:], wout[:])

            # ---- phase 1: gT/uT = W.T @ xT, aT = silu(gT)*uT ----
            for b in range(NB):
                cap = caps[b]
                off = offs[b]
                chunks = _token_chunks(cap)
                starts = [off + sum(chunks[:i]) for i in range(len(chunks))]
                for fg in range(NFG):
                    if b == 0 and fg in pre:
                        wgb, wub = pre[fg]
                    else:
                        wgb = w1pool.tile([P, KD, P * FG], bf16, tag="wgb",
                                          name=f"wgb_{b}_{fg}")
                        wub = w1pool.tile([P, KD, P * FG], bf16, tag="wub",
                                          name=f"wub_{b}_{fg}")
                        nc.gpsimd.dma_start(wgb[:, :kh, :], wg_d[b, fg][:, :kh, :])
                        nc.gpsimd.dma_start(wgb[:, kh:, :], wg_d[b, fg][:, kh:, :])
                        nc.gpsimd.dma_start(wub[:, :kh, :], wu_d[b, fg][:, :kh, :])
                        nc.gpsimd.dma_start(wub[:, kh:, :], wu_d[b, fg][:, kh:, :])

                    for fs in range(FG):
                        ft = fg * FG + fs
                        for ci, (c0, cn) in enumerate(zip(starts, chunks)):
                            pg = ppool.tile([P, cn], f32, tag="pg")
                            pu = ppool.tile([P, cn], f32, tag="pu")
                            for ko in range(KD):
                                nc.tensor.matmul(
                                    pg[:], wgb[:, ko, fs * P:(fs + 1) * P],
                                    xt[:, ko, c0:c0 + cn],
                                    start=(ko == 0), stop=(ko == KD - 1))
                            for ko in range(KD):
                                nc.tensor.matmul(
                                    pu[:], wub[:, ko, fs * P:(fs + 1) * P],
                                    xt[:, ko, c0:c0 + cn],
                                    start=(ko == 0), stop=(ko == KD - 1))
                            a_sl = at[:, ft, c0:c0 + cn]
                            nc.scalar.activation(
                                a_sl, pg[:], mybir.ActivationFunctionType.Silu)
                            nc.vector.tensor_tensor(a_sl, a_sl, pu[:], mult)

            # ---- phase 2: dT = Wd.T @ aT, out = dT * w ----
            for b in range(NB):
                cap = caps[b]
                off = offs[b]
                chunks = _token_chunks(cap)
                starts = [off + sum(chunks[:i]) for i in range(len(chunks))]
                for dg in range(NDG):
                    pds = [[ppool.tile([P, cn], f32, tag=f"pd{ds}c{ci}",
                                       name=f"pd_{b}_{dg}_{ds}_{ci}")
                            for ci, cn in enumerate(chunks)]
                           for ds in range(DG)]
                    for fb in range(NFB):
                        wdb = wd2pool.tile([P, KO2, P * DG], bf16, tag="wdb")
                        nc.gpsimd.dma_start(wdb[:], wd_d[b, dg, fb])
                        for ko in range(KO2):
                            fk = fb * KO2 + ko
                            for ds in range(DG):
                                for ci, (c0, cn) in enumerate(zip(starts, chunks)):
                                    nc.tensor.matmul(
                                        pds[ds][ci][:],
                                        wdb[:, ko, ds * P:(ds + 1) * P],
                                        at[:, fk, c0:c0 + cn],
                                        start=(fk == 0), stop=(fk == KF - 1))
                    for ds in range(DG):
                        ot = outpool.tile([P, cap], bf16, tag="ot")
                        for ci, (c0, cn) in enumerate(zip(starts, chunks)):
                            nc.vector.tensor_tensor(
                                ot[:, c0 - off:c0 - off + cn], pds[ds][ci][:],
                                wrep[:, c0:c0 + cn], mult)
                        dt_idx = dg * DG + ds
                        nc.sync.dma_start(
                            out_d[dt_idx * P:(dt_idx + 1) * P, off:off + cap],
                            ot[:])

    nc.compile()
    return nc


def _swizzle_w1(w):
    """[D, F] -> [NFG, P, KD, P*FG] block-major, partition-contiguous."""
    import ml_dtypes
    return np.ascontiguousarray(
        w.reshape(KD, P, NFG, P * FG).transpose(2, 1, 0, 3)
        .astype(ml_dtypes.bfloat16))


def _swizzle_wd(w):
    """[F, D] -> [NDG, NFB, P, KO2, P*DG] block-major."""
    import ml_dtypes
    return np.ascontiguousarray(
        w.reshape(NFB, KO2, P, NDG, P * DG).transpose(3, 0, 2, 1, 4)
        .astype(ml_dtypes.bfloat16))


def kernel(x, gate_tensor, Wg, Wu, Wd):
    global LAST_RESULT
    import ml_dtypes
    from concourse.bass_interp import get_hw_module
    from concourse.bass_utils import run_bass_kernel_spmd

    bf16 = ml_dtypes.bfloat16
    x = np.ascontiguousarray(np.asarray(x, dtype=np.float32))
    gate_tensor = np.asarray(gate_tensor, dtype=np.float32)
    Wg = np.asarray(Wg, dtype=np.float32)
    Wu = np.asarray(Wu, dtype=np.float32)
    Wd = np.asarray(Wd, dtype=np.float32)

    # ---- router (replicated; tiny: T*D*E flops) ----
    logits = x @ gate_tensor                      # [T, E] fp32
    m = logits.max(axis=-1, keepdims=True)
    p = np.exp(logits - m, dtype=np.float32)
    p /= p.sum(axis=-1, keepdims=True)
    topi = np.argsort(-p, axis=-1, kind="stable")[:, :TOPK]      # [T, K]
    topw = np.take_along_axis(p, topi, axis=-1)
    topw = topw / (topw.sum(axis=-1, keepdims=True) + 1e-20)

    idx = []          # tokens routed to each expert
    wts = []          # their combine weights
    for e in range(E):
        sel = (topi == e)                         # [T, K]; <=1 True per row
        idx.append(np.nonzero(sel.any(axis=-1))[0])
        wts.append(topw[sel].astype(np.float32))  # row-major == token order

    # ---- slot balancing ----
    # Each core runs two SPMD-uniform blocks (capA, capB).  An expert is
    # assigned either two A-slots on two cores (AA), two B-slots (BB), or
    # one A + one B slot (AB, hosted on a single core).  With k experts AA
    # (the k largest), k BB (the k smallest) and 8-2k AB, per-core slots =
    # capA + capB = ceil(c_max/2) + max(ceil(c_BB/2), c_AB - capA); pick
    # the k that minimizes it (518 for the target routing vs 546 naive).
    order = sorted(range(E), key=lambda e: -len(idx[e]))
    cnt = [len(idx[e]) for e in order]
    best = None
    for k in range(E // 2 + 1):
        if k == 0:
            ca = (cnt[0] + 1) // 2
            cb = cnt[0] - ca
        else:
            ca = (max(cnt[:k]) + 1) // 2
            cb = max((max(cnt[E - k:]) + 1) // 2,
                     max([c - ca for c in cnt[k:E - k]] or [0]))
        ca, cb = max(ca, 2), max(cb, 2)
        if best is None or ca + cb < best[0] + best[1]:
            best = (ca, cb, k)
    capA, capB, k = best
    caps = (capA, capB)

    # slot assignment: (coreA, coreB) per expert; splits sized to caps
    slotA = [None] * E      # expert whose tokens fill core c's A block
    slotB = [None] * E
    for i in range(k):
        slotA[2 * i], slotA[2 * i + 1] = order[i], order[i]       # AA
        slotB[2 * i], slotB[2 * i + 1] = order[E - 1 - i], order[E - 1 - i]
    for m, e in enumerate(order[k:E - k]):                        # AB
        c = 2 * k + m
        slotA[c] = e
        slotB[c] = e

    if caps not in _COMPILED:
        _COMPILED[caps] = _build(caps)
    nc = _COMPILED[caps]

    # ---- dispatch: per-core inputs (pre-swizzled to SBUF block layout) ----
    capT = capA + capB
    swz = {e: (_swizzle_w1(Wg[e]), _swizzle_w1(Wu[e]), _swizzle_wd(Wd[e]))
           for e in range(E)}
    taken = [0] * E         # tokens of expert e already placed
    in_maps = []
    halves = []             # per core: [(tok_idx, n_tokens, col_off)]
    for c in range(E):
        eA, eB = slotA[c], slotB[c]
        xt = np.zeros((P, KD, capT), dtype=bf16)
        wr = np.zeros((P, capT), dtype=np.float32)
        segs = []
        for (e, off, cap) in ((eA, 0, capA), (eB, capA, capB)):
            lo = taken[e]
            n = min(cap, len(idx[e]) - lo)
            taken[e] = lo + n
            tok = idx[e][lo:lo + n]
            if n:
                xt[:, :, off:off + n] = (
                    x[tok].T.reshape(KD, P, n).transpose(1, 0, 2).astype(bf16))
                wr[:, off:off + n] = wts[e][lo:lo + n][None, :]
            segs.append((tok, n, off))
        halves.append(segs)
        in_maps.append({"xt": xt,
                        "wg": np.stack([swz[eA][0], swz[eB][0]]),
                        "wu": np.stack([swz[eA][1], swz[eB][1]]),
                        "wd": np.stack([swz[eA][2], swz[eB][2]]),
                        "wrep": wr})
    assert all(taken[e] == len(idx[e]) for e in range(E)), (taken, caps)

    trace = bool(int(os.environ.get("KERNEL_TRACE", "0")))
    old_m = nc.m
    nc.m = get_hw_module(nc.m)
    try:
        try:
            res = run_bass_kernel_spmd(nc, in_maps, core_ids=list(range(E)),
                                       trace=trace)
        except (ImportError, ModuleNotFoundError):
            # tracing requested (e.g. BASS_TRACE in the env) but this image
            # lacks the axon NTFF profile hook -- rerun without tracing
            os.environ["BASS_NEVER_TRACE"] = "1"
            res = run_bass_kernel_spmd(nc, in_maps, core_ids=list(range(E)),
                                       trace=False)
    finally:
        nc.m = old_m
    LAST_RESULT = res

    # ---- combine: scatter-add the per-core partials ----
    out = np.zeros((T, D), dtype=np.float32)
    for core in range(E):
        ot = res.results[core]["out_t"]
        for tok, n, off in halves[core]:
            if n:
                out[tok] += ot[:, off:off + n].T.astype(np.float32)
    return out

